# revision 12
# baseline (speedup 1.0000x reference)
"""Trainium2 Bass kernel for the ActorSNN problem.

Network (reference semantics, T=8 steps, all fp32):
    x_in = state @ W_in.T + b_in                       # constant across steps
    per step:
        r1   = (mem1 - th1 > 0)          (detached)
        mem1 = clip(b1,0,1)*mem1 + x_in - r1*th1
        s1   = (mem1 - th1 > 0)
        h    = s1 @ W_h.T + b_h
        r2   = (mem2 - th2 > 0)
        mem2 = clip(b2,0,1)*mem2 + h - r2*th2
        s2   = (mem2 - th2 > 0)
        ssum += s2
    out = tanh((ssum/8) @ W_out.T + b_out)             # [B, 1]

Distribution: pure data-parallel. B=8192 is sharded 1024/core across 8
NeuronCores; all weights replicated; each core computes its [1024] slice of
the output which the host concatenates.

Numerical strategy (the system is chaotic -- threshold crossings amplify
matmul rounding into discrete spike flips, so matmul precision is critical):
  * x_in via bf16x6-style limb decomposition: state and W_in are each split
    into bf16 limbs (s0+s1+s2, w0+w1+w2); the 6 dominant cross products are
    accumulated in fp32 PSUM.  bf16*bf16 products are exact in fp32, so this
    reproduces the fp32 x_in to ~1e-8.
  * Spikes are exactly representable in bf16 ({0, th}); W_h is split into 2
    bf16 limbs (weight error ~2^-18), giving h accurate to ~1.5e-6 -- enough
    that end-to-end l2 error vs the fp32 reference is ~1e-3 (measured).
  * The elementwise LIF recurrence is computed in fp32 on DVE with the same
    association order as the reference, so given identical x_in the layer-1
    membrane trajectory is bit-exact.

Engine mapping per step (per batch-half of 512):
  PE : h accumulation (2 limb matmuls x 8 K-chunks per output chunk), the
       reset fold (diag(-1) @ th*s2 accumulated into the same PSUM bank),
       and the running W_out matvec on the step's spikes.
  DVE: mem updates as fused scalar_tensor_tensor (mem*beta + x), spike
       generation as fused tensor_scalar (mem > th)*th.
"""

import os
import numpy as np
import ml_dtypes

from contextlib import ExitStack

import concourse.bass as bass
import concourse.mybir as mybir
import concourse.tile as tile
from concourse import bacc
from concourse.bass_utils import run_bass_kernel_spmd
from concourse.masks import make_identity

bf16 = ml_dtypes.bfloat16
F32 = mybir.dt.float32
BF16 = mybir.dt.bfloat16

NCORES = 8
B, S, H, T = 8192, 256, 1024, 8
BC = B // NCORES          # 1024 batch rows per core
NH = 2                    # batch halves per core (SBUF footprint)
BH = BC // NH             # 512
C = H // 128              # 8 H-chunks
SC = S // 128             # 2 S-chunks

LAST_RESULT = {}


def _split_limbs(w, n):
    """Split fp32 array into n bf16 limbs (w ~= sum of limbs)."""
    w = np.asarray(w, np.float32)
    limbs = []
    rem = w
    for _ in range(n):
        hi = rem.astype(bf16)
        limbs.append(hi)
        rem = rem - hi.astype(np.float32)
    return limbs


def build_nc():
    # debug bisect flags
    T_ = int(os.environ.get("SNN_T", T))
    NH_ = int(os.environ.get("SNN_NH", NH))
    no_l1 = os.environ.get("SNN_NO_L1", "0") == "1"
    no_l2 = os.environ.get("SNN_NO_L2", "0") == "1"
    no_diag = os.environ.get("SNN_NO_DIAG", "0") == "1"
    no_matvec = os.environ.get("SNN_NO_MATVEC", "0") == "1"
    dummy_out = no_matvec or no_l2 or T_ == 0

    nc = bacc.Bacc(
        "TRN2",
        target_bir_lowering=False,
        debug=False,
        num_devices=NCORES,
    )

    d_state = nc.declare_dram_parameter("stateT", [S, BC], F32, isOutput=False)
    d_wi = [nc.declare_dram_parameter(f"wi{i}", [S, H], BF16, isOutput=False)
            for i in range(3)]
    d_wh = [nc.declare_dram_parameter(f"wh{i}", [H, H], BF16, isOutput=False)
            for i in range(2)]
    d_wmv = [nc.declare_dram_parameter(f"wmv{i}", [H], BF16, isOutput=False)
             for i in range(2)]
    d_beta1 = nc.declare_dram_parameter("beta1", [H], F32, isOutput=False)
    d_th1 = nc.declare_dram_parameter("th1", [H], F32, isOutput=False)
    d_b1 = nc.declare_dram_parameter("b1", [H], F32, isOutput=False)
    d_beta2 = nc.declare_dram_parameter("beta2", [H], F32, isOutput=False)
    d_th2 = nc.declare_dram_parameter("th2", [H], F32, isOutput=False)
    d_bout = nc.declare_dram_parameter("bout", [1], F32, isOutput=False)
    d_diag = nc.declare_dram_parameter("diagm", [128, 128], BF16,
                                       isOutput=False)
    d_out = nc.declare_dram_parameter("out", [1, BC], F32, isOutput=True)

    ag = mybir.AluOpType.is_gt
    amul = mybir.AluOpType.mult
    aadd = mybir.AluOpType.add
    asub = mybir.AluOpType.subtract
    amax = mybir.AluOpType.max
    amin = mybir.AluOpType.min

    with tile.TileContext(nc) as tc, ExitStack() as ctx:
        consts = ctx.enter_context(tc.tile_pool(name="consts", bufs=1))
        xinp = ctx.enter_context(tc.tile_pool(name="xin", bufs=2))
        memp = ctx.enter_context(tc.tile_pool(name="mem", bufs=1))
        s1p = ctx.enter_context(tc.tile_pool(name="s1", bufs=2))
        s2p = ctx.enter_context(tc.tile_pool(name="s2", bufs=1))
        prep32 = ctx.enter_context(tc.tile_pool(name="prep32", bufs=2))
        prep16 = ctx.enter_context(tc.tile_pool(name="prep16", bufs=6))
        ysb = ctx.enter_context(tc.tile_pool(name="ysb", bufs=2))
        psum = ctx.enter_context(tc.tile_pool(name="psum", bufs=4, space="PSUM"))
        ypsum = ctx.enter_context(tc.tile_pool(name="ypsum", bufs=2, space="PSUM"))

        # ---- constants ----
        wi = [consts.tile([128, SC, H], BF16, name=f"wi{i}", tag=f"wi{i}") for i in range(3)]
        for i in range(3):
            for kc in range(SC):
                nc.sync.dma_start(out=wi[i][:, kc, :],
                                  in_=d_wi[i][kc * 128:(kc + 1) * 128, :])
        wh = [consts.tile([128, C, H], BF16, name=f"wh{i}", tag=f"wh{i}") for i in range(2)]
        for i in range(2):
            for kc in range(C):
                nc.sync.dma_start(out=wh[i][:, kc, :],
                                  in_=d_wh[i][kc * 128:(kc + 1) * 128, :])
        wmv = [consts.tile([128, C, 1], BF16, name=f"wmv{i}", tag=f"wmv{i}") for i in range(2)]
        for i in range(2):
            nc.sync.dma_start(
                out=wmv[i][:, :, 0],
                in_=d_wmv[i].ap().rearrange("(c p) -> p c", p=128))

        def vec_tile(d, tag):
            t = consts.tile([128, C], F32, name=tag, tag=tag)
            nc.sync.dma_start(out=t, in_=d.ap().rearrange("(c p) -> p c", p=128))
            return t

        beta1v = vec_tile(d_beta1, "beta1")
        th1v = vec_tile(d_th1, "th1")
        b1v = vec_tile(d_b1, "b1")
        beta2v = vec_tile(d_beta2, "beta2")
        th2v = vec_tile(d_th2, "th2")
        # clip(beta, 0, 1) in place
        nc.vector.tensor_scalar(beta1v, beta1v, 0.0, 1.0, amax, amin)
        nc.vector.tensor_scalar(beta2v, beta2v, 0.0, 1.0, amax, amin)

        bout_sb = consts.tile([1, 1], F32, name="bout_sb", tag="bout")
        nc.sync.dma_start(out=bout_sb,
                          in_=d_bout.ap().rearrange("(p o) -> p o", p=1))

        # negated identity for the reset fold: psum += (-I) @ (th*s2)
        diagm = consts.tile([128, 128], BF16, name="diagm", tag="diagm")
        nc.sync.dma_start(out=diagm, in_=d_diag.ap())

        # persistent state (shared by both halves; halves run sequentially)
        mem1 = memp.tile([128, C, BH], F32, name="mem1", tag="mem1")
        mem2 = memp.tile([128, C, BH], F32, name="mem2", tag="mem2")
        ths1 = [s1p.tile([128, C, BH], BF16, name=f"ths1_{i}", tag=f"ths1_{i}") for i in range(2)]
        ths2 = s2p.tile([128, C, BH], BF16, name="ths2", tag="ths2")

        for half in range(NH_):
            bsl = slice(half * BH, (half + 1) * BH)

            # ---- split state into 3 bf16 limbs (on device) ----
            st32 = prep32.tile([128, SC, BH], F32, name="st32", tag="st32")
            for kc in range(SC):
                nc.sync.dma_start(out=st32[:, kc, :],
                                  in_=d_state[kc * 128:(kc + 1) * 128, bsl])
            slimb = [prep16.tile([128, SC, BH], BF16, name=f"sl{i}", tag=f"sl{i}")
                     for i in range(3)]
            nc.vector.tensor_copy(slimb[0][:], st32[:])
            nc.vector.tensor_sub(st32[:], st32[:], slimb[0][:])
            nc.vector.tensor_copy(slimb[1][:], st32[:])
            nc.vector.tensor_sub(st32[:], st32[:], slimb[1][:])
            nc.vector.tensor_copy(slimb[2][:], st32[:])

            # ---- x_in = state @ W_in.T + b_in via 6 limb products ----
            x_in = xinp.tile([128, C, BH], F32, name="x_in", tag="xin")
            for j in range(C):
                ps = psum.tile([128, BH], F32, name="ps", tag="ps")
                first = True
                for (a, w) in ((0, 0), (0, 1), (1, 0), (1, 1), (0, 2), (2, 0)):
                    for kc in range(SC):
                        last = (a, w, kc) == (2, 0, SC - 1)
                        nc.tensor.matmul(
                            ps[:], wi[w][:, kc, j * 128:(j + 1) * 128],
                            slimb[a][:, kc, :], start=first, stop=last)
                        first = False
                nc.vector.tensor_scalar(
                    x_in[:, j, :], ps[:], b1v[:, j:j + 1], None, aadd)

            # ---- init membranes and initial resets ----
            nc.vector.memset(mem1[:], 0.0)
            nc.vector.memset(mem2[:], 0.0)
            for j in range(C):
                nc.vector.tensor_scalar(
                    ths1[1][:, j, :], mem1[:, j, :],
                    th1v[:, j:j + 1], th1v[:, j:j + 1], ag, amul)
                nc.vector.tensor_scalar(
                    ths2[:, j, :], mem2[:, j, :],
                    th2v[:, j:j + 1], th2v[:, j:j + 1], ag, amul)

            yps = ypsum.tile([1, BH], F32, name="yps", tag="yps")

            for t in range(T_):
                s_prev = ths1[(t + 1) % 2]
                s_cur = ths1[t % 2]
                # ---- layer 1 (DVE only, fp32, reference association order) --
                for j in range(C if not no_l1 else 0):
                    # mem1 = (beta1*mem1) + x_in
                    nc.vector.scalar_tensor_tensor(
                        mem1[:, j, :], mem1[:, j, :], beta1v[:, j:j + 1],
                        x_in[:, j, :], amul, aadd)
                    # mem1 -= th1*s1_prev   (exact: values {0, th})
                    nc.vector.tensor_sub(
                        mem1[:, j, :], mem1[:, j, :], s_prev[:, j, :])
                    # spike: (mem1 > th1) * th1 -> bf16
                    nc.vector.tensor_scalar(
                        s_cur[:, j, :], mem1[:, j, :],
                        th1v[:, j:j + 1], th1v[:, j:j + 1], ag, amul)

                # ---- layer 2 ----
                for j in range(C if not no_l2 else 0):
                    ps = psum.tile([128, BH], F32, name="ps", tag="ps")
                    for li in range(2):
                        for kc in range(C):
                            nc.tensor.matmul(
                                ps[:], wh[li][:, kc, j * 128:(j + 1) * 128],
                                s_cur[:, kc, :],
                                start=(li == 0 and kc == 0), stop=False)
                    # reset fold: ps += (-I) @ (th2*s2_prev)
                    if not no_diag:
                        nc.tensor.matmul(ps[:], diagm[:], ths2[:, j, :],
                                         start=False, stop=True)
                    else:
                        nc.tensor.matmul(ps[:], wh[0][:, 0, j * 128:(j + 1) * 128],
                                         ths2[:, j, :], start=False, stop=True)
                    # mem2 = (beta2*mem2) + (h - th2*s2_prev)
                    nc.vector.scalar_tensor_tensor(
                        mem2[:, j, :], mem2[:, j, :], beta2v[:, j:j + 1],
                        ps[:], amul, aadd)
                    # spike2: (mem2 > th2) * th2 -> bf16
                    nc.vector.tensor_scalar(
                        ths2[:, j, :], mem2[:, j, :],
                        th2v[:, j:j + 1], th2v[:, j:j + 1], ag, amul)
                    # running output matvec: yps += (W_out/(8*th2)) @ (th2*s2)
                    if not no_matvec:
                        for li in range(2):
                            nc.tensor.matmul(
                                yps[:], wmv[li][:, j, :], ths2[:, j, :],
                                start=(t == 0 and j == 0 and li == 0),
                                stop=(t == T_ - 1 and j == C - 1 and li == 1))

            # ---- out = tanh(y + b_out) ----
            if dummy_out:
                nc.sync.dma_start(out=d_out[0:1, bsl], in_=x_in[0:1, 0, :])
            else:
                y_sb = ysb.tile([1, BH], F32, name="y_sb", tag="ysb")
                nc.scalar.activation(y_sb[:], yps[:],
                                     mybir.ActivationFunctionType.Tanh,
                                     bias=bout_sb[:, :], scale=1.0)
                nc.sync.dma_start(out=d_out[0:1, bsl], in_=y_sb[0:1, :])

    nc.compile()
    return nc


_NC_CACHE = {}


def _get_nc():
    if "nc" not in _NC_CACHE:
        _NC_CACHE["nc"] = build_nc()
    return _NC_CACHE["nc"]


def prepare_in_maps(state, W_in, b_in, beta_in, th_in, W_h, b_h, beta_h,
                    th_h, W_out, b_out):
    state = np.ascontiguousarray(np.asarray(state, np.float32))
    W_in = np.asarray(W_in, np.float32)
    W_h = np.asarray(W_h, np.float32)
    W_out = np.asarray(W_out, np.float32)
    th_in = np.asarray(th_in, np.float32)
    th_h = np.asarray(th_h, np.float32)
    b_h = np.asarray(b_h, np.float32)
    assert np.all(b_h == 0.0), "kernel assumes b_h == 0 (reference uses zeros)"

    # host-side weight layout prep (transposes, limb encoding)
    wi_l = [np.ascontiguousarray(w) for w in _split_limbs(W_in.T, 3)]
    # fold 1/th1 into W_h so the matmul can consume th1*s1 directly
    whT = np.ascontiguousarray(W_h.T / th_in[:, None])
    wh_l = [np.ascontiguousarray(w) for w in _split_limbs(whT, 2)]
    # fold the /T rate normalization and 1/th2 into W_out
    wmv = W_out[0] / (np.float32(T) * th_h)
    wmv_l = _split_limbs(wmv, 2)

    stateT = np.ascontiguousarray(state.T)  # [S, B]

    in_maps = []
    for ci in range(NCORES):
        sl = slice(ci * BC, (ci + 1) * BC)
        in_maps.append({
            "stateT": np.ascontiguousarray(stateT[:, sl]),
            "wi0": wi_l[0], "wi1": wi_l[1], "wi2": wi_l[2],
            "wh0": wh_l[0], "wh1": wh_l[1],
            "wmv0": wmv_l[0], "wmv1": wmv_l[1],
            "beta1": np.asarray(beta_in, np.float32),
            "th1": th_in, "b1": np.asarray(b_in, np.float32),
            "beta2": np.asarray(beta_h, np.float32), "th2": th_h,
            "bout": np.asarray(b_out, np.float32).reshape(1),
            "diagm": -np.eye(128, dtype=bf16),
        })
    return in_maps


def kernel(**inputs):
    in_maps = prepare_in_maps(**inputs)
    nc = _get_nc()
    res = run_bass_kernel_spmd(nc, in_maps, core_ids=list(range(NCORES)))
    LAST_RESULT["exec_time_ns"] = res.exec_time_ns
    out = np.concatenate([np.asarray(res.results[ci]["out"]).ravel()
                          for ci in range(NCORES)])
    return out.reshape(B, 1).astype(np.float32)
